# revision 1
# baseline (speedup 1.0000x reference)
"""Trainium2 Bass kernel for Conv2dWeightModulate (StyleGAN2-style modulated conv).

Math restructure 1 (modulation): the per-sample modulated conv
    out[b] = conv(conv_w * c * style[b,cin] * sigma_inv[b,cout], x_pad[b])
is rewritten as
    out[b,cout] = sigma_inv[b,cout] * conv(conv_w, (x[b] * c*style[b,cin])_pad)
so the conv weights are sample-independent (resident in SBUF) and the
per-sample modulation becomes a per-input-channel scale of x plus a
per-output-channel scale of the result. sigma has the closed form
    sigma^2[b,cout] = c^2 * sum_cin style[b,cin]^2 * sum_k conv_w[cout,cin,k]^2
computed on host (tiny [B,CIN] x [CIN,COUT] product), as is the 3-layer
mapping network producing style (all [16,512]-sized, <0.01% of FLOPs).

Math restructure 2 (Winograd F(2,3) along the height axis): each pair of
output rows (2p, 2p+1) is computed from 4 transformed input rows
    v0 = d[2p]-d[2p+2], v1 = d[2p+1]+d[2p+2],
    v2 = d[2p+2]-d[2p+1], v3 = d[2p+1]-d[2p+3]
with host-transformed weights U = G @ w over the kh axis
    (G = [[1,0,0],[.5,.5,.5],[.5,-.5,.5],[0,0,1]])
and output rows  out[2p] = M0+M1+M2,  out[2p+1] = M1-M2-M3  where
    M[pos] = sum_{cin,kw} U[pos,kw] * v[pos]  (shifted by kw).
This cuts tensor-engine MACs by 1.5x (12 accumulation steps per output
tile instead of 18 row-equivalents); the width axis stays direct (3 taps
against a replicate-padded 66-wide SBUF image).

Device: data-parallel over batch, 2 samples per core on 8 cores, fp16
operands (PE at 1 col/cycle, fp32 PSUM accumulate). Per (sample, half,
cout-block, row-chunk): 4 PSUM banks hold M[0..3] for 8 row-pairs x 64
cols; 48 accumulating 128x128 @ 128x512 matmuls fill them; ScalarE applies
sigma_inv during PSUM->SBUF eviction and VectorE forms the two output-row
combinations in fp32. A per-sample power-of-2 prescale keeps scaled x in
fp16's normal range and is undone exactly in the fp32 output scale.
"""

import numpy as np
from contextlib import ExitStack

import concourse.bass as bass
import concourse.tile as tile
from concourse import bacc, mybir
from concourse import bass_utils

B, CIN, COUT, KS, H, W, DLAT = 16, 512, 512, 3, 64, 64, 512
EPS = 1e-8
N_CORES = 8
SPC = B // N_CORES          # samples per core
NCB = CIN // 128            # cin blocks
NOB = COUT // 128           # cout blocks
HALF = H // 2               # rows per half-image
NPAIR = HALF // 2           # winograd row-pairs per half (16)
PPC = 8                     # row-pairs per PSUM chunk (8 pairs * 64 = 512)
NCH = NPAIR // PPC          # chunks per half (2)
PADW = W + 2
PADR = HALF + 2
NPOS = 4                    # winograd positions
_cache = {}


def _build():
    if "nc" in _cache:
        return _cache["nc"]
    f32 = mybir.dt.float32
    f16 = mybir.dt.float16
    nc = bacc.Bacc("TRN2", target_bir_lowering=False, debug=False,
                   num_devices=N_CORES)
    x_d = nc.dram_tensor("x", [SPC, CIN, H, W], f16, kind="ExternalInput").ap()
    # U[cb, p, pos, kw, cout]
    wt_d = nc.dram_tensor("wt", [NCB, 128, NPOS, KS, COUT], f16,
                          kind="ExternalInput").ap()
    sig_d = nc.dram_tensor("sig", [128, SPC, NOB], f32,
                           kind="ExternalInput").ap()
    out_d = nc.dram_tensor("out", [SPC, COUT, H * W], f32,
                           kind="ExternalOutput").ap()

    with tile.TileContext(nc) as tc, ExitStack() as ctx:
        cpool = ctx.enter_context(tc.tile_pool(name="const", bufs=1))
        stpool = ctx.enter_context(tc.tile_pool(name="stage", bufs=4))
        vpool = ctx.enter_context(tc.tile_pool(name="v", bufs=2))
        smpool = ctx.enter_context(tc.tile_pool(name="sm", bufs=8))
        opool = ctx.enter_context(tc.tile_pool(name="o", bufs=8))
        pspool = ctx.enter_context(tc.tile_pool(name="ps", bufs=8, space="PSUM"))

        wt_sb = cpool.tile([128, NCB, NPOS, KS, COUT], f16)
        sig_sb = cpool.tile([128, SPC, NOB], f32)

        # PE pre-warm: ~4us of dummy matmuls while the first DMAs land, so
        # the HAM clock-gate is already at 8/8 when real matmuls start
        warm_t = cpool.tile([128, 128], f16)
        warm_ps = pspool.tile([128, PPC * W], f32, name="warm_ps", tag="ps")
        nc.gpsimd.memset(warm_t[:], 0.0)
        for _ in range(80):
            nc.tensor.matmul(warm_ps[:, 0:64], warm_t[:], warm_t[:, 0:64],
                             start=True, stop=True)

        nrows = HALF + 1  # 33 original rows cover one half
        for s in range(SPC):
            for hf in range(2):
                # V is built straight from a contiguous staging copy of the
                # original rows; replicate-padding commutes with the (linear)
                # row transform, so V's column pads are plain copies and the
                # one clamped boundary row becomes a single-pair fixup.
                v_t = vpool.tile([128, NCB, NPOS, NPAIR, PADW], f16)
                r0 = 0 if hf == 0 else HALF - 1
                for cb in range(NCB):
                    st = stpool.tile([128, nrows * W], f16, name="st", tag="st")
                    nc.sync.dma_start(
                        st[:],
                        x_d[s, cb * 128:(cb + 1) * 128,
                            r0:r0 + nrows, :].rearrange("c a b -> c (a b)"))
                    if s == 0 and hf == 0:
                        # weight loads interleaved after each x block so
                        # cin-block cb's matmuls start as soon as possible
                        nc.sync.dma_start(wt_sb[:, cb], wt_d[cb])
                    sr = st.rearrange("c (a b) -> c a b", b=W)
                    # winograd input transform over row pairs (d = padded
                    # rows): v0=d0-d2, v1=d1+d2, v2=d2-d1, v3=d1-d3
                    vi = v_t[:, cb]
                    if hf == 0:
                        # d_i = sr[i-1] for i>=1, d_0 = sr[0] (clamped row)
                        m = slice(1, NPAIR)      # pairs 1..15 regular
                        nc.vector.tensor_sub(vi[:, 0, m, 1:W + 1],
                                             sr[:, 1:31:2], sr[:, 3:33:2])
                        nc.vector.tensor_add(vi[:, 1, m, 1:W + 1],
                                             sr[:, 2:32:2], sr[:, 3:33:2])
                        nc.vector.tensor_sub(vi[:, 2, m, 1:W + 1],
                                             sr[:, 3:33:2], sr[:, 2:32:2])
                        nc.vector.tensor_sub(vi[:, 3, m, 1:W + 1],
                                             sr[:, 2:32:2], sr[:, 4:33:2])
                        # pair 0 fixup: d0=d1=sr0, d2=sr1, d3=sr2
                        nc.vector.tensor_sub(vi[:, 0, 0:1, 1:W + 1],
                                             sr[:, 0:1], sr[:, 1:2])
                        nc.vector.tensor_add(vi[:, 1, 0:1, 1:W + 1],
                                             sr[:, 0:1], sr[:, 1:2])
                        nc.vector.tensor_sub(vi[:, 2, 0:1, 1:W + 1],
                                             sr[:, 1:2], sr[:, 0:1])
                        nc.vector.tensor_sub(vi[:, 3, 0:1, 1:W + 1],
                                             sr[:, 0:1], sr[:, 2:3])
                    else:
                        # d_i = sr[i] for i<=32, d_33 = sr[32] (clamped row)
                        nc.vector.tensor_sub(vi[:, 0, :, 1:W + 1],
                                             sr[:, 0:32:2], sr[:, 2:33:2])
                        nc.vector.tensor_add(vi[:, 1, :, 1:W + 1],
                                             sr[:, 1:33:2], sr[:, 2:33:2])
                        nc.vector.tensor_sub(vi[:, 2, :, 1:W + 1],
                                             sr[:, 2:33:2], sr[:, 1:33:2])
                        m = slice(0, NPAIR - 1)  # pairs 0..14 regular
                        nc.vector.tensor_sub(vi[:, 3, m, 1:W + 1],
                                             sr[:, 1:31:2], sr[:, 3:33:2])
                        # pair 15 fixup: d31=sr31, d33=sr32 (clamped)
                        nc.vector.tensor_sub(vi[:, 3, NPAIR - 1:NPAIR, 1:W + 1],
                                             sr[:, 31:32], sr[:, 32:33])
                    # V column pads (replicate: padded col0==col1, 65==64)
                    nc.vector.tensor_copy(vi[:, :, :, 0], vi[:, :, :, 1])
                    nc.vector.tensor_copy(vi[:, :, :, W + 1], vi[:, :, :, W])
                if s == 0 and hf == 0:
                    nc.sync.dma_start(sig_sb[:], sig_d[:])

                for ob in range(NOB):
                    for ch in range(NCH):
                        pts = [pspool.tile([128, PPC * W], f32,
                                           name="ps", tag="ps")
                               for _ in range(NPOS)]
                        # pos pair (0,1) accumulates fully before (2,3) so
                        # their evictions overlap the second half's matmuls;
                        # cin-block-major so the first MMs only need block 0
                        for pp in (0, 2):
                            for cb in range(NCB):
                                for kw in range(KS):
                                    for pos in (pp, pp + 1):
                                        lhsT = wt_sb[:, cb, pos, kw,
                                                     ob * 128:(ob + 1) * 128]
                                        rhs = v_t[:, cb, pos,
                                                  ch * PPC:(ch + 1) * PPC,
                                                  kw:kw + W]
                                        nc.tensor.matmul(
                                            pts[pos][:], lhsT, rhs,
                                            start=(cb == 0 and kw == 0),
                                            stop=(cb == NCB - 1 and kw == KS - 1))
                        # sigma_inv scale on ScalarE (doubles as PSUM evict)
                        sms = []
                        for pos in range(NPOS):
                            sm = smpool.tile([128, PPC * W], f32,
                                             name="sm", tag="sm")
                            nc.scalar.mul(sm[:], pts[pos][:],
                                          sig_sb[:, s, ob:ob + 1])
                            sms.append(sm)
                        # output rows: even = m0+m1+m2, odd = m1-m2-m3
                        te = opool.tile([128, PPC * W], f32, name="te", tag="t")
                        oe = opool.tile([128, PPC * W], f32, name="oe", tag="oo")
                        to = opool.tile([128, PPC * W], f32, name="to", tag="t")
                        oo = opool.tile([128, PPC * W], f32, name="oo", tag="oo")
                        nc.vector.tensor_add(te[:], sms[0][:], sms[1][:])
                        nc.vector.tensor_add(oe[:], te[:], sms[2][:])
                        nc.vector.tensor_sub(to[:], sms[1][:], sms[2][:])
                        nc.vector.tensor_sub(oo[:], to[:], sms[3][:])
                        row0 = hf * HALF + ch * PPC * 2
                        dst = out_d[s, ob * 128:(ob + 1) * 128,
                                    row0 * W:(row0 + 2 * PPC) * W]
                        dst = dst.rearrange("c (p two w) -> c p two w",
                                            two=2, w=W)
                        oe3 = oe.rearrange("c (p w) -> c p w", w=W)
                        oo3 = oo.rearrange("c (p w) -> c p w", w=W)
                        nc.sync.dma_start(dst[:, :, 0, :], oe3[:])
                        nc.sync.dma_start(dst[:, :, 1, :], oo3[:])
    nc.compile()
    _cache["nc"] = nc
    return nc


def _prelu(z, a):
    return np.where(z >= 0, z, a * z)


_G = np.array([[1.0, 0.0, 0.0],
               [0.5, 0.5, 0.5],
               [0.5, -0.5, 0.5],
               [0.0, 0.0, 1.0]], dtype=np.float32)


def _prepare(inputs):
    x = np.asarray(inputs["x"], dtype=np.float32)
    s = np.asarray(inputs["s"], dtype=np.float32)
    map_w0 = np.asarray(inputs["map_w0"], dtype=np.float32)
    map_b0 = np.asarray(inputs["map_b0"], dtype=np.float32)
    a0 = np.asarray(inputs["prelu_a0"], dtype=np.float32)
    map_w1 = np.asarray(inputs["map_w1"], dtype=np.float32)
    map_b1 = np.asarray(inputs["map_b1"], dtype=np.float32)
    a1 = np.asarray(inputs["prelu_a1"], dtype=np.float32)
    style_w = np.asarray(inputs["style_w"], dtype=np.float32)
    style_b = np.asarray(inputs["style_b"], dtype=np.float32)
    conv_w = np.asarray(inputs["conv_w"], dtype=np.float32)

    c_lin = np.float32(1.0 / np.sqrt(DLAT))
    z = _prelu(s @ (map_w0 * c_lin).T + map_b0, a0)
    z = _prelu(z @ (map_w1 * c_lin).T + map_b1, a1)
    style = z @ (style_w * c_lin).T + style_b          # [B, CIN]

    c_conv = 1.0 / np.sqrt(CIN * KS * KS)
    w2 = ((conv_w.astype(np.float64) * c_conv) ** 2).sum(axis=(2, 3))  # [COUT, CIN]
    sig2 = (style.astype(np.float64) ** 2) @ w2.T                      # [B, COUT]
    sig_inv = (1.0 / np.sqrt(sig2 + EPS)).astype(np.float32)
    msc = (style * np.float32(c_conv)).astype(np.float32)              # [B, CIN]

    # per-sample power-of-2 normalizer keeps msc*x in fp16's normal range;
    # undone exactly in the fp32 output scale
    rms = np.sqrt(np.mean((msc.astype(np.float64)) ** 2, axis=1)) + 1e-30
    k = np.clip(np.round(-np.log2(rms)), -20, 40).astype(np.int32)     # [B]
    pw = np.exp2(k.astype(np.float32))                                  # 2^k
    msc_n = msc * pw[:, None]
    sig_n = sig_inv / pw[:, None]

    # fold the per-cin style scale into x on host, cast fp16
    x_scaled = (x * msc_n[:, :, None, None]).astype(np.float16)

    # winograd weight transform over kh: U[pos] = sum_kh G[pos,kh] w[..,kh,..]
    # conv_w: [COUT, CIN, KH, KW] -> U: [COUT, CIN, NPOS, KW]
    u = np.einsum("pk,ockw->ocpw", _G, conv_w).astype(np.float16)
    # -> [NCB, 128, NPOS, KW, COUT]
    wt_host = np.ascontiguousarray(
        u.reshape(COUT, NCB, 128, NPOS, KS).transpose(1, 2, 3, 4, 0))

    sig_r = sig_n.reshape(B, NOB, 128)
    in_maps = []
    for c in range(N_CORES):
        sl = slice(c * SPC, (c + 1) * SPC)
        in_maps.append({
            "x": np.ascontiguousarray(x_scaled[sl]),
            "wt": wt_host,
            "sig": np.ascontiguousarray(sig_r[sl].transpose(2, 0, 1)),
        })
    return in_maps


def run(inputs, **spmd_kwargs):
    nc = _build()
    in_maps = _prepare(inputs)
    res = bass_utils.run_bass_kernel_spmd(
        nc, in_maps, core_ids=list(range(N_CORES)), **spmd_kwargs)
    out = np.concatenate(
        [res.results[c]["out"].reshape(SPC, COUT, H, W)
         for c in range(N_CORES)], axis=0)
    return out, res


def kernel(**inputs) -> np.ndarray:
    out, _ = run(inputs)
    return out



# revision 5
# speedup vs baseline: 1.2583x; 1.2583x over previous
"""Trainium2 Bass kernel for Conv2dWeightModulate (StyleGAN2-style modulated conv).

Math restructure 1 (modulation): the per-sample modulated conv
    out[b] = conv(conv_w * c * style[b,cin] * sigma_inv[b,cout], x_pad[b])
is rewritten as
    out[b,cout] = sigma_inv[b,cout] * conv(conv_w, (x[b] * c*style[b,cin])_pad)
so the conv weights are sample-independent (resident in SBUF) and the
per-sample modulation becomes a per-input-channel scale of x plus a
per-output-channel scale of the result. sigma has the closed form
    sigma^2[b,cout] = c^2 * sum_cin style[b,cin]^2 * sum_k conv_w[cout,cin,k]^2
computed on host (tiny [B,CIN] x [CIN,COUT] product), as is the 3-layer
mapping network producing style (all [16,512]-sized, <0.01% of FLOPs).

Math restructure 2 (Winograd F(4,3) along the height axis): each quad of
output rows (4q..4q+3) is computed from 6 transformed input rows
    v0 = 4d0-5d2+d4, v1 = -4(d1+d2)+(d3+d4), v2 = 4(d1-d2)+(d4-d3),
    v3 = 2(d3-d1)+(d4-d2), v4 = -2(d3-d1)+(d4-d2), v5 = 4d1-5d3+d5
with host-transformed weights U = G @ w over the kh axis
    (G = [[1/4,0,0],[-1/6,-1/6,-1/6],[-1/6,1/6,-1/6],
          [1/24,1/12,1/6],[1/24,-1/12,1/6],[0,0,1]])
and output rows
    out0 = m0+m1+m2+m3+m4,  out1 = m1-m2+2(m3-m4),
    out2 = (m1+m2)+4(m3+m4), out3 = m1-m2+8(m3-m4)+m5
where M[pos] = sum_{cin,kw} U[pos,kw] * v[pos] (shifted by kw).
This cuts tensor-engine MACs 2x vs direct conv (18 accumulation steps per
8 output rows instead of 36); the width axis stays direct (3 taps against
a replicate-padded 66-wide SBUF image).

Device: data-parallel over batch, 2 samples per core on 8 cores, fp16
operands (PE at 1 col/cycle, fp32 PSUM accumulate). Per (sample, chunk of
8 quads, cout-block): 6 PSUM banks hold M[0..5] for 8 quads x 64 cols; 72
accumulating 128x128 @ 128x512 matmuls fill them pos-major; VectorE forms
the four output-row combinations in fp32 (using fused scalar_tensor_tensor
for the x2/x4/x8 taps), ScalarE applies sigma_inv, and the input row
transform for the NEXT chunk is interleaved into the vector queue between
combine groups so the tensor engine never waits on it. A per-sample
power-of-2 prescale keeps scaled x in fp16's normal range and is undone
exactly in the fp32 output scale.
"""

import numpy as np
from contextlib import ExitStack

import concourse.bass as bass
import concourse.tile as tile
from concourse import bacc, mybir
from concourse import bass_utils

B, CIN, COUT, KS, H, W, DLAT = 16, 512, 512, 3, 64, 64, 512
EPS = 1e-8
N_CORES = 8
SPC = B // N_CORES          # samples per core
NCB = CIN // 128            # cin blocks
NOB = COUT // 128           # cout blocks
NPOS = 6                    # winograd F(4,3) positions
NQ = H // 4                 # row quads per sample (16)
QPC = 8                     # quads per PSUM chunk (8 quads * 64 = 512)
NCH = NQ // QPC             # chunks per sample (2)
PADW = W + 2
NROWS = 33                  # original rows staged per chunk
_cache = {}

_MUL = mybir.AluOpType.mult
_ADD = mybir.AluOpType.add


def _build():
    if "nc" in _cache:
        return _cache["nc"]
    f32 = mybir.dt.float32
    f16 = mybir.dt.float16
    nc = bacc.Bacc("TRN2", target_bir_lowering=False, debug=False,
                   num_devices=N_CORES)
    x_d = nc.dram_tensor("x", [SPC, CIN, H, W], f16, kind="ExternalInput").ap()
    # U[pos, cb, 128, kw, cout]
    wt_d = nc.dram_tensor("wt", [NPOS, NCB, 128, KS, COUT], f16,
                          kind="ExternalInput").ap()
    sig_d = nc.dram_tensor("sig", [128, SPC, NOB], f32,
                           kind="ExternalInput").ap()
    out_d = nc.dram_tensor("out", [SPC, COUT, H * W], f32,
                           kind="ExternalOutput").ap()

    with tile.TileContext(nc) as tc, ExitStack() as ctx:
        cpool = ctx.enter_context(tc.tile_pool(name="const", bufs=1))
        stpool = ctx.enter_context(tc.tile_pool(name="stage", bufs=4))
        vpool = ctx.enter_context(tc.tile_pool(name="v", bufs=2))
        vtpool = ctx.enter_context(tc.tile_pool(name="vt", bufs=8))
        ctpool = ctx.enter_context(tc.tile_pool(name="ct", bufs=12))
        opool = ctx.enter_context(tc.tile_pool(name="o", bufs=4))
        epool = ctx.enter_context(tc.tile_pool(name="e", bufs=4))
        pspool = ctx.enter_context(tc.tile_pool(name="ps", bufs=8, space="PSUM"))

        wt_sb = cpool.tile([128, NCB, NPOS, KS, COUT], f16)
        sig_sb = cpool.tile([128, SPC, NOB], f32)

        # PE pre-warm: dummy matmuls while the first DMAs land, so the HAM
        # clock-gate is already at 8/8 when real matmuls start
        warm_t = cpool.tile([128, 128], f16)
        warm_ps = pspool.tile([128, QPC * W], f32, name="warm_ps", tag="ps")
        nc.gpsimd.memset(warm_t[:], 0.0)
        for _ in range(80):
            nc.tensor.matmul(warm_ps[:, 0:64], warm_t[:], warm_t[:, 0:64],
                             start=True, stop=True)

        st_tiles = {}

        def emit_st_dmas(s, ch):
            # stage the 33 original rows covering chunk ch of sample s
            r0 = 0 if ch == 0 else H - NROWS
            for cb in range(NCB):
                st = stpool.tile([128, NROWS * W], f16, name="st", tag="st")
                nc.sync.dma_start(
                    st[:],
                    x_d[s, cb * 128:(cb + 1) * 128,
                        r0:r0 + NROWS, :].rearrange("c a b -> c (a b)"))
                st_tiles[(s, ch, cb)] = st

        def v_tile(s, ch):
            key = ("v", s, ch)
            if key not in _cache_v:
                _cache_v[key] = vpool.tile([128, NCB, NPOS, QPC, PADW], f16,
                                           name=f"v{s}{ch}", tag="v")
            return _cache_v[key]

        _cache_v = {}

        def emit_vbuild_group(s, ch, group):
            """group 0: pos 0,1; group 1: pos 2,3; group 2: pos 4,5.

            d_i views: chunk0 uses d_i = st[4q+i-1] (pad row 0 duplicated,
            so only v0's q=0 needs a fixup); chunk1 uses d_i = st[4q+i]
            (pad row 65 duplicated, so only v5's q=7 needs a fixup).
            Replicate-padding commutes with the (linear) row transform, so
            V's column pads are plain copies.
            """
            v_t = v_tile(s, ch)
            off = -1 if ch == 0 else 0
            for cb in range(NCB):
                st = st_tiles[(s, ch, cb)]
                sr = st.rearrange("c (a b) -> c a b", b=W)

                def d(i, q0=0, nq=QPC):
                    lo = 4 * q0 + i + off
                    return sr[:, lo:lo + 4 * (nq - 1) + 1:4]

                vi = v_t[:, cb]
                if group == 0:
                    # pos0: v0 = 4 d0 - 5 d2 + d4
                    g1 = vtpool.tile([128, QPC, W], f16, name="g1", tag="vt")
                    nc.vector.scalar_tensor_tensor(
                        g1[:], d(2), -5.0, d(4), _MUL, _ADD)
                    if ch == 0:
                        nc.vector.scalar_tensor_tensor(
                            vi[:, 0, 1:, 1:W + 1], d(0, q0=1, nq=QPC - 1),
                            4.0, g1[:, 1:], _MUL, _ADD)
                        # q=0 fixup: d0 = clamped row = sr[0]
                        nc.vector.scalar_tensor_tensor(
                            vi[:, 0, 0:1, 1:W + 1], sr[:, 0:1], 4.0,
                            g1[:, 0:1], _MUL, _ADD)
                    else:
                        nc.vector.scalar_tensor_tensor(
                            vi[:, 0, :, 1:W + 1], d(0), 4.0, g1[:], _MUL, _ADD)
                    # pos1: v1 = -4(d1+d2) + (d3+d4)
                    s12 = vtpool.tile([128, QPC, W], f16, name="s12", tag="vt")
                    t34 = vtpool.tile([128, QPC, W], f16, name="t34", tag="vt")
                    nc.vector.tensor_add(s12[:], d(1), d(2))
                    nc.vector.tensor_add(t34[:], d(3), d(4))
                    nc.vector.scalar_tensor_tensor(
                        vi[:, 1, :, 1:W + 1], s12[:], -4.0, t34[:], _MUL, _ADD)
                    pads = slice(0, 2)
                elif group == 1:
                    # pos2: v2 = 4(d1-d2) + (d4-d3)
                    m12 = vtpool.tile([128, QPC, W], f16, name="m12", tag="vt")
                    n43 = vtpool.tile([128, QPC, W], f16, name="n43", tag="vt")
                    nc.vector.tensor_sub(m12[:], d(1), d(2))
                    nc.vector.tensor_sub(n43[:], d(4), d(3))
                    nc.vector.scalar_tensor_tensor(
                        vi[:, 2, :, 1:W + 1], m12[:], 4.0, n43[:], _MUL, _ADD)
                    # pos3: v3 = 2(d3-d1) + (d4-d2)
                    a = vtpool.tile([128, QPC, W], f16, name="a", tag="vt")
                    bb = vtpool.tile([128, QPC, W], f16, name="b", tag="vt")
                    nc.vector.tensor_sub(a[:], d(3), d(1))
                    nc.vector.tensor_sub(bb[:], d(4), d(2))
                    nc.vector.scalar_tensor_tensor(
                        vi[:, 3, :, 1:W + 1], a[:], 2.0, bb[:], _MUL, _ADD)
                    # pos4: v4 = -2(d3-d1) + (d4-d2)
                    nc.vector.scalar_tensor_tensor(
                        vi[:, 4, :, 1:W + 1], a[:], -2.0, bb[:], _MUL, _ADD)
                    pads = slice(2, 5)
                else:
                    # pos5: v5 = 4 d1 - 5 d3 + d5
                    g2 = vtpool.tile([128, QPC, W], f16, name="g2", tag="vt")
                    if ch == 0:
                        nc.vector.scalar_tensor_tensor(
                            g2[:], d(3), -5.0, d(5), _MUL, _ADD)
                        nc.vector.scalar_tensor_tensor(
                            vi[:, 5, :, 1:W + 1], d(1), 4.0, g2[:], _MUL, _ADD)
                    else:
                        nc.vector.scalar_tensor_tensor(
                            g2[:, 0:QPC - 1], d(3, nq=QPC - 1), -5.0,
                            d(5, nq=QPC - 1), _MUL, _ADD)
                        # q=7 fixup: d5 = clamped row = sr[32]
                        nc.vector.scalar_tensor_tensor(
                            g2[:, QPC - 1:QPC], sr[:, 31:32], -5.0,
                            sr[:, 32:33], _MUL, _ADD)
                        nc.vector.scalar_tensor_tensor(
                            vi[:, 5, :, 1:W + 1], d(1), 4.0, g2[:], _MUL, _ADD)
                    pads = slice(5, 6)
                # V column pads (replicate: padded col0==col1, 65==64) on
                # the otherwise-idle scalar engine
                nc.scalar.copy(vi[:, pads, :, 0], vi[:, pads, :, 1])
                nc.scalar.copy(vi[:, pads, :, W + 1], vi[:, pads, :, W])

        def emit_tileset(s, ch, ob):
            v_t = v_tile(s, ch)
            pts = [pspool.tile([128, QPC * W], f32, name="ps", tag="ps")
                   for _ in range(NPOS)]
            ct = {}

            def combine(nm, fn):
                t = ctpool.tile([128, QPC * W], f32, name=nm, tag="ct")
                fn(t)
                ct[nm] = t
                return t

            outs = [opool.tile([128, QPC * W], f32, name=f"o{i}", tag="o")
                    for i in range(4)]
            for pos in range(NPOS):
                for cb in range(NCB):
                    for kw in range(KS):
                        nc.tensor.matmul(
                            pts[pos][:],
                            wt_sb[:, cb, pos, kw, ob * 128:(ob + 1) * 128],
                            v_t[:, cb, pos, :, kw:kw + W],
                            start=(cb == 0 and kw == 0),
                            stop=(cb == NCB - 1 and kw == KS - 1))
                # vector TT ops may read at most ONE PSUM operand, so m1/m3
                # go through ScalarE copies before the two-M combines
                if pos == 1:
                    combine("e1", lambda t: nc.scalar.copy(t[:], pts[1][:]))
                elif pos == 2:
                    # p=m1+m2, q=m1-m2, u=m0+p
                    combine("p", lambda t: nc.vector.tensor_add(
                        t[:], ct["e1"][:], pts[2][:]))
                    combine("q", lambda t: nc.vector.tensor_sub(
                        t[:], ct["e1"][:], pts[2][:]))
                    combine("u", lambda t: nc.vector.tensor_add(
                        t[:], ct["p"][:], pts[0][:]))
                elif pos == 3:
                    combine("e3", lambda t: nc.scalar.copy(t[:], pts[3][:]))
                elif pos == 4:
                    # r=m3+m4, t=m3-m4, then all but out3
                    combine("r", lambda t: nc.vector.tensor_add(
                        t[:], ct["e3"][:], pts[4][:]))
                    combine("t", lambda t: nc.vector.tensor_sub(
                        t[:], ct["e3"][:], pts[4][:]))
                    nc.vector.tensor_add(outs[0][:], ct["u"][:], ct["r"][:])
                    nc.vector.scalar_tensor_tensor(
                        outs[1][:], ct["t"][:], 2.0, ct["q"][:], _MUL, _ADD)
                    nc.vector.scalar_tensor_tensor(
                        outs[2][:], ct["r"][:], 4.0, ct["p"][:], _MUL, _ADD)
                    combine("s3", lambda t: nc.vector.scalar_tensor_tensor(
                        t[:], ct["t"][:], 8.0, ct["q"][:], _MUL, _ADD))
            nc.vector.tensor_add(outs[3][:], ct["s3"][:], pts[5][:])
            # sigma_inv scale on ScalarE, then DMA rows 4q+i of the chunk
            row0 = ch * QPC * 4
            dst = out_d[s, ob * 128:(ob + 1) * 128,
                        row0 * W:(row0 + 4 * QPC) * W]
            dst = dst.rearrange("c (p four w) -> c p four w", four=4, w=W)
            for i in range(4):
                oe = epool.tile([128, QPC * W], f32, name=f"e{i}", tag="e")
                nc.scalar.mul(oe[:], outs[i][:], sig_sb[:, s, ob:ob + 1])
                nc.sync.dma_start(
                    dst[:, :, i, :], oe.rearrange("c (p w) -> c p w", w=W))

        # weight DMAs pos-major so the first tile-set's pos-0 matmuls can
        # start after ~1/6 of the weight traffic has landed
        def emit_wt_dmas(pos):
            for cb in range(NCB):
                nc.sync.dma_start(wt_sb[:, cb, pos], wt_d[pos, cb])

        chunks = [(s, ch) for s in range(SPC) for ch in range(NCH)]
        emit_st_dmas(*chunks[0])
        emit_wt_dmas(0)
        emit_wt_dmas(1)
        nc.sync.dma_start(sig_sb[:], sig_d[:])
        for g in range(3):
            emit_vbuild_group(*chunks[0], g)
            if g < 2:
                emit_wt_dmas(2 * g + 2)
                emit_wt_dmas(2 * g + 3)
        for ci, (s, ch) in enumerate(chunks):
            for ob in range(NOB):
                emit_tileset(s, ch, ob)
                # interleave the next chunk's input transform into the
                # vector queue so it overlaps this chunk's matmuls
                if ci + 1 < len(chunks) and ob < 3:
                    if ob == 0:
                        emit_st_dmas(*chunks[ci + 1])
                    emit_vbuild_group(*chunks[ci + 1], ob)
    nc.compile()
    _cache["nc"] = nc
    return nc


def _prelu(z, a):
    return np.where(z >= 0, z, a * z)


_G = np.array([[1 / 4, 0, 0],
               [-1 / 6, -1 / 6, -1 / 6],
               [-1 / 6, 1 / 6, -1 / 6],
               [1 / 24, 1 / 12, 1 / 6],
               [1 / 24, -1 / 12, 1 / 6],
               [0, 0, 1]], dtype=np.float64)


def _prepare(inputs):
    x = np.asarray(inputs["x"], dtype=np.float32)
    s = np.asarray(inputs["s"], dtype=np.float32)
    map_w0 = np.asarray(inputs["map_w0"], dtype=np.float32)
    map_b0 = np.asarray(inputs["map_b0"], dtype=np.float32)
    a0 = np.asarray(inputs["prelu_a0"], dtype=np.float32)
    map_w1 = np.asarray(inputs["map_w1"], dtype=np.float32)
    map_b1 = np.asarray(inputs["map_b1"], dtype=np.float32)
    a1 = np.asarray(inputs["prelu_a1"], dtype=np.float32)
    style_w = np.asarray(inputs["style_w"], dtype=np.float32)
    style_b = np.asarray(inputs["style_b"], dtype=np.float32)
    conv_w = np.asarray(inputs["conv_w"], dtype=np.float32)

    c_lin = np.float32(1.0 / np.sqrt(DLAT))
    z = _prelu(s @ (map_w0 * c_lin).T + map_b0, a0)
    z = _prelu(z @ (map_w1 * c_lin).T + map_b1, a1)
    style = z @ (style_w * c_lin).T + style_b          # [B, CIN]

    c_conv = 1.0 / np.sqrt(CIN * KS * KS)
    w2 = ((conv_w.astype(np.float64) * c_conv) ** 2).sum(axis=(2, 3))  # [COUT, CIN]
    sig2 = (style.astype(np.float64) ** 2) @ w2.T                      # [B, COUT]
    sig_inv = (1.0 / np.sqrt(sig2 + EPS)).astype(np.float32)
    msc = (style * np.float32(c_conv)).astype(np.float32)              # [B, CIN]

    # per-sample power-of-2 normalizer keeps msc*x in fp16's normal range;
    # undone exactly in the fp32 output scale
    rms = np.sqrt(np.mean((msc.astype(np.float64)) ** 2, axis=1)) + 1e-30
    k = np.clip(np.round(-np.log2(rms)), -20, 40).astype(np.int32)     # [B]
    pw = np.exp2(k.astype(np.float32))                                  # 2^k
    msc_n = msc * pw[:, None]
    sig_n = sig_inv / pw[:, None]

    # fold the per-cin style scale into x on host, cast fp16
    x_scaled = (x * msc_n[:, :, None, None]).astype(np.float16)

    # winograd F(4,3) weight transform over kh: U[pos] = sum_kh G[pos,kh] w
    # conv_w: [COUT, CIN, KH, KW] -> U: [NPOS, NCB, 128, KW, COUT]
    u = np.einsum("pk,ockw->ocpw", _G, conv_w.astype(np.float64)).astype(np.float16)
    wt_host = np.ascontiguousarray(
        u.reshape(COUT, NCB, 128, NPOS, KS).transpose(3, 1, 2, 4, 0))

    sig_r = sig_n.reshape(B, NOB, 128)
    in_maps = []
    for c in range(N_CORES):
        sl = slice(c * SPC, (c + 1) * SPC)
        in_maps.append({
            "x": np.ascontiguousarray(x_scaled[sl]),
            "wt": wt_host,
            "sig": np.ascontiguousarray(sig_r[sl].transpose(2, 0, 1)),
        })
    return in_maps


def run(inputs, **spmd_kwargs):
    nc = _build()
    in_maps = _prepare(inputs)
    res = bass_utils.run_bass_kernel_spmd(
        nc, in_maps, core_ids=list(range(N_CORES)), **spmd_kwargs)
    out = np.concatenate(
        [res.results[c]["out"].reshape(SPC, COUT, H, W)
         for c in range(N_CORES)], axis=0)
    return out, res


def kernel(**inputs) -> np.ndarray:
    out, _ = run(inputs)
    return out
